# revision 1
# baseline (speedup 1.0000x reference)
"""Trainium2 Bass kernel for nn_NegUniform (topk_masking).

Computes: L2-normalize feature & negative_features, sims = f_hat @ negs_hat^T
per negative set j (masked same-class for j==idx), top-16 per row, softmax
entropy over the J axis, decay-weighted mean + log(J).

Sharding: data-parallel over the n (row) dimension of `feature` across 8
NeuronCores; negative_features / target replicated. Each core returns
per-row-group partial sums [128, RT]; the host reduces them to the scalar.

Host-side prep (layout/quantization only): normalize + bf16-cast + transpose
of feature and negatives, one-hot mask tables, decay table.

Per-core pipeline (PE and DVE co-saturated; the max8 top-k scan is the
hard floor at ~1 elem/lane/cycle from PSUM):
  - j processed with idx LAST so the pipeline start is not gated on the
    mask tables and the mask-matmul weight-grouping happens mid-stream.
  - negsT[j] [D, N] bf16 and fT [D, n_local] bf16 DMA'd over 3 queues
    (sync/scalar HWDGE + gpsimd SWDGE); the first-processed j is split
    into 4 pieces across all queues so matmuls start ~11us in; activation
    tables (Ln then Exp) warmed during the load phase.
  - per (row-tile, j): 4 chunks of 1024 cands; 2 bf16 matmuls
    [128x128]@[128x512] per chunk into a PSUM tile (4 tiles = all 8 banks
    in flight); same-class mask for j==idx folded in as a rank-4 one-hot
    matmul accumulated into the same PSUM bank.
  - top-16 per row: DVE max8 per 1024-chunk directly from PSUM (union of
    chunk top-8s = 32 cands), then max8 + match_replace + max8.
  - entropy: per-tile numerators only (A = sum_j e_j*d_j, S = sum_j e_j,
    d_j = v_j - max_j v_j) with TT chains on GpSimd (last tile on Vector)
    and Exp on Scalar, so the Vector queue never blocks on cross-engine
    chains mid-scan; one batched epilogue (reciprocal, Ln, combine,
    reduce) computes (A/S - T*lnS) * decay/T for all tiles at the end.
"""

import math
import sys

import numpy as np

for _p in ("/opt/trn_rl_repo",):
    if _p not in sys.path:
        sys.path.insert(0, _p)

N = 4096
D = 128
J = 4
NCORES = 8
NLOC = N // NCORES          # 512 rows per core
RT = NLOC // 128            # 4 row-tiles per core
K = 16
TEMP = 0.01
V = 0.95
MASK_NEG = -448.0           # dominates any cosine sim
CHUNK = 1024                # candidates per PSUM tile / max8 scan
NCHUNK = N // CHUNK

_BUILD_CACHE = {}
LAST_RESULT = None  # BassKernelResults of the most recent kernel() call


def _build(idx: int):
    if idx in _BUILD_CACHE:
        return _BUILD_CACHE[idx]

    import concourse.bacc as bacc
    import concourse.tile as tile
    import concourse.mybir as mybir

    f32 = mybir.dt.float32
    e4m3 = mybir.dt.bfloat16
    AF = mybir.ActivationFunctionType
    OP = mybir.AluOpType

    nc = bacc.Bacc(
        "TRN2",
        target_bir_lowering=False,
        debug=False,
        enable_asserts=False,
        num_devices=NCORES,
    )

    fTd = nc.dram_tensor("fT", [D, NLOC], e4m3, kind="ExternalInput").ap()
    negsTd = nc.dram_tensor("negsT", [J, D, N], e4m3, kind="ExternalInput").ap()
    maskLd = nc.dram_tensor("maskL", [J, NLOC], e4m3, kind="ExternalInput").ap()
    onehotd = nc.dram_tensor("onehotR", [J, N], e4m3, kind="ExternalInput").ap()
    decayd = nc.dram_tensor("decayW", [128, RT * K], f32,
                            kind="ExternalInput").ap()
    outd = nc.dram_tensor("out", [128, RT], f32, kind="ExternalOutput").ap()

    with tile.TileContext(nc) as tc:
        with (
            tc.tile_pool(name="consts", bufs=1) as cpool,
            tc.tile_pool(name="negs", bufs=1) as npool,
            tc.tile_pool(name="cands", bufs=4) as capool,
            tc.tile_pool(name="ent", bufs=2) as epool,
            tc.tile_pool(name="psums", bufs=4, space="PSUM") as psp,
        ):
            # j processing order: idx LAST, so the start of the pipeline is
            # not gated on the mask tables and the mask-matmul serialization
            # happens mid-stream when the DVE has plenty of queued work.
            jorder = [j for j in range(J) if j != idx] + [idx]

            # ---- loads: first-j across all 4 queues in 4 pieces, rest
            # spread so every tensor lands well before its first use ----
            fT = cpool.tile([128, NLOC], e4m3)
            nc.scalar.dma_start(fT, fTd)
            decay_t = cpool.tile([128, RT * K], f32)
            nc.gpsimd.dma_start(decay_t, decayd)

            negs_t = {}
            H = N // 2
            for j in range(J):
                negs_t[j] = npool.tile([128, N], e4m3, tag=f"negsT{j}",
                                       name=f"negsT{j}")
            j0 = jorder[0]
            Q = N // 4
            # first chunk's columns as a 512-col piece so the first matmul
            # is gated on the fewest possible bytes
            nc.sync.dma_start(negs_t[j0][:, 0:512], negsTd[j0, :, 0:512])
            nc.sync.dma_start(negs_t[j0][:, 512:Q], negsTd[j0, :, 512:Q])
            for c, eng in ((1, nc.scalar), (2, nc.gpsimd), (3, nc.sync)):
                eng.dma_start(negs_t[j0][:, c * Q:(c + 1) * Q],
                              negsTd[j0, :, c * Q:(c + 1) * Q])
            onehot_t = cpool.tile([J, N], e4m3)
            nc.scalar.dma_start(onehot_t, onehotd)
            maskL_t = cpool.tile([J, NLOC], e4m3)
            nc.scalar.dma_start(maskL_t, maskLd)
            for j, eng in ((jorder[1], nc.sync), (jorder[2], nc.gpsimd),
                           (jorder[3], nc.scalar)):
                for h in range(2):
                    eng.dma_start(
                        negs_t[j][:, h * H:(h + 1) * H],
                        negsTd[j, :, h * H:(h + 1) * H],
                    )

            # Warm the Exp activation table AFTER all DMA triggers: the
            # warm-up runs on the scalar ENGINE queue, and placing it
            # earlier blocks the scalar queue's DMA triggers behind the
            # decay-DMA wait plus a 1.28us table load. Emitted here it
            # executes during the load phase, well before the first real
            # Exp. (No Ln warm-up: Exp evicts it before the epilogue
            # anyway, so the epilogue pays that single reload regardless.)
            warm = cpool.tile([128, 8], f32)
            nc.scalar.activation(out=warm, in_=decay_t[:, 0:8], func=AF.Exp)

            partials = cpool.tile([128, RT], f32)
            Sall = cpool.tile([128, RT * K], f32)
            Aall = cpool.tile([128, RT * K], f32)

            # ---- main loop: sims chunks -> max8 union -> top16 ----
            Vt = {}
            for t in range(RT):
                Vt[t] = cpool.tile([128, J * K], f32, tag=f"V{t}",
                                   name=f"V{t}")
            for t in range(RT):
                for j in jorder:
                    cand = capool.tile([128, 8 * NCHUNK], f32, tag="cand")
                    for c in range(NCHUNK):
                        ps = psp.tile([128, CHUNK], f32, tag="sims")
                        for h in range(CHUNK // 512):
                            m0 = c * CHUNK + h * 512
                            nc.tensor.matmul(
                                ps[:, h * 512:(h + 1) * 512],
                                lhsT=fT[:, t * 128:(t + 1) * 128],
                                rhs=negs_t[j][:, m0:m0 + 512],
                                start=True, stop=(j != idx),
                            )
                        if j == idx:
                            for h in range(CHUNK // 512):
                                m0 = c * CHUNK + h * 512
                                nc.tensor.matmul(
                                    ps[:, h * 512:(h + 1) * 512],
                                    lhsT=maskL_t[:, t * 128:(t + 1) * 128],
                                    rhs=onehot_t[:, m0:m0 + 512],
                                    start=False, stop=True,
                                )
                        nc.vector.max(out=cand[:, c * 8:(c + 1) * 8], in_=ps)
                    top8 = Vt[t][:, j * K:j * K + 8]
                    nc.vector.max(out=top8, in_=cand)
                    rep = capool.tile([128, 8 * NCHUNK], f32, tag="rep")
                    nc.vector.match_replace(
                        out=rep, in_to_replace=top8, in_values=cand,
                        imm_value=-1e30,
                    )
                    nc.vector.max(out=Vt[t][:, j * K + 8:j * K + 16], in_=rep)

                # ---- entropy numerators for tile t ----
                # ent_t/T * decay = (A/S - T*lnS) * decay/T with
                # A = sum_j e_j*d_j, S = sum_j e_j, e_j = exp(d_j/T),
                # d_j = v_j - max_j v_j  (uses sum_j p_j = 1).
                # In-loop: only cheap maxes on Vector (no cross-engine
                # stalls), TT chains on GpSimd (last tile on Vector, which
                # is idle by then), Exp on Scalar.  The reciprocal/Ln/
                # combine runs once, batched over all tiles, at the end.
                eng = nc.vector if t == RT - 1 else nc.gpsimd
                v_ = [Vt[t][:, j * K:(j + 1) * K] for j in range(J)]
                m01 = epool.tile([128, K], f32, tag="m01", name=f"m01_{t}")
                m23 = epool.tile([128, K], f32, tag="m23", name=f"m23_{t}")
                m = epool.tile([128, K], f32, tag="m", name=f"m_{t}")
                nc.vector.tensor_tensor(m01, v_[0], v_[1], op=OP.max)
                nc.vector.tensor_tensor(m23, v_[2], v_[3], op=OP.max)
                nc.vector.tensor_tensor(m, m01, m23, op=OP.max)
                d_ = [epool.tile([128, K], f32, tag=f"d{j}", name=f"d{j}_{t}")
                      for j in range(J)]
                e_ = [epool.tile([128, K], f32, tag=f"e{j}", name=f"e{j}_{t}")
                      for j in range(J)]
                for j in range(J):
                    eng.tensor_tensor(d_[j], v_[j], m, op=OP.subtract)
                    nc.scalar.activation(out=e_[j], in_=d_[j], func=AF.Exp,
                                         scale=1.0 / TEMP)
                sl = slice(t * K, (t + 1) * K)
                eng.tensor_tensor(Sall[:, sl], e_[0], e_[1], op=OP.add)
                eng.tensor_tensor(Sall[:, sl], Sall[:, sl], e_[2], op=OP.add)
                eng.tensor_tensor(Sall[:, sl], Sall[:, sl], e_[3], op=OP.add)
                for j in range(J):
                    eng.tensor_tensor(e_[j], e_[j], d_[j], op=OP.mult)
                eng.tensor_tensor(e_[0], e_[0], e_[1], op=OP.add)
                eng.tensor_tensor(e_[2], e_[2], e_[3], op=OP.add)
                eng.tensor_tensor(Aall[:, sl], e_[0], e_[2], op=OP.add)

            # ---- batched epilogue over all tiles: [128, RT*K] ops ----
            W = RT * K
            rS = cpool.tile([128, W], f32)
            nc.vector.reciprocal(rS, Sall)
            lnS = cpool.tile([128, W], f32)
            nc.scalar.activation(out=lnS, in_=Sall, func=AF.Ln)
            nc.vector.tensor_tensor(Aall, Aall, rS, op=OP.mult)  # A/S
            # negacc = T*lnS - A/S; escr = negacc * (-decay/T)
            nc.vector.scalar_tensor_tensor(
                out=Aall, in0=lnS, scalar=TEMP, in1=Aall,
                op0=OP.mult, op1=OP.subtract,
            )
            nc.vector.tensor_tensor(Aall, Aall, decay_t, op=OP.mult)
            nc.vector.tensor_reduce(
                out=partials, in_=Aall.rearrange("p (t k) -> p t k", k=K),
                op=OP.add, axis=mybir.AxisListType.X,
            )

            nc.sync.dma_start(outd, partials)

    nc.compile()
    _BUILD_CACHE[idx] = nc
    return nc


def kernel(feature, target, negative_features, idx):
    import ml_dtypes
    from concourse.bass_utils import run_bass_kernel_spmd

    e4m3 = ml_dtypes.bfloat16

    feature = np.asarray(feature, dtype=np.float32)
    target = np.asarray(target).astype(np.int64)
    negs = np.asarray(negative_features, dtype=np.float32)
    idx_i = int(np.asarray(idx))

    # normalize + cast + transpose on host (layout/quantization prep)
    f = feature / np.maximum(
        np.linalg.norm(feature, axis=-1, keepdims=True), 1e-12)
    g = negs / np.maximum(
        np.linalg.norm(negs, axis=-1, keepdims=True), 1e-12)
    fT_all = np.ascontiguousarray(f.T.astype(e4m3))                # [D, N]
    negsT = np.ascontiguousarray(g.transpose(0, 2, 1).astype(e4m3))  # [J,D,N]
    onehot = (target[None, :] == np.arange(J)[:, None])
    onehotR = np.ascontiguousarray(onehot.astype(e4m3))            # [J, N]
    maskL_full = (MASK_NEG * onehot.astype(np.float32)).astype(e4m3)
    decay = (V ** np.arange(K, dtype=np.float64))
    decay = decay / decay.sum()
    decay_row = np.tile((-decay / TEMP).astype(np.float32), RT)  # [RT*K]
    decayW = np.broadcast_to(decay_row, (128, RT * K)).copy()

    nc = _build(idx_i)
    in_maps = []
    for c in range(NCORES):
        sl = slice(c * NLOC, (c + 1) * NLOC)
        in_maps.append({
            "fT": np.ascontiguousarray(fT_all[:, sl]),
            "negsT": negsT,
            "maskL": np.ascontiguousarray(maskL_full[:, sl]),
            "onehotR": onehotR,
            "decayW": decayW,
        })

    res = run_bass_kernel_spmd(nc, in_maps, core_ids=list(range(NCORES)))
    global LAST_RESULT
    LAST_RESULT = res
    total = 0.0
    for c in range(NCORES):
        total += float(np.asarray(res.results[c]["out"], dtype=np.float64).sum())
    loss = total / N + math.log(J)
    return np.float32(loss)


if __name__ == "__main__":
    rng = np.random.default_rng(0)
    f = rng.standard_normal((N, D)).astype(np.float32)
    ng = rng.standard_normal((J, N, D)).astype(np.float32)
    tg = rng.integers(0, J, size=N).astype(np.int64)
    print(kernel(f, tg, ng, 0))



# revision 5
# speedup vs baseline: 1.0605x; 1.0605x over previous
"""Trainium2 Bass kernel for nn_NegUniform (topk_masking) — v3.

Computes: L2-normalize feature & negative_features, sims = f_hat @ negs_hat^T
per negative set j (masked same-class for j==idx), top-16 per row, softmax
entropy over the J axis, decay-weighted mean + log(J).

Sharding: data-parallel over the n (row) dimension across 8 NeuronCores;
negatives/targets replicated. Each core returns per-row-tile partial sums
[128, RT]; the host reduces to the scalar.

Design (engine-measured rates drove every choice):
  - PE: fp8(e4m3) matmuls in DoubleRow perf mode (2 k-tiles summed per
    pass, 0.5 cyc/col). For j!=idx the 2nd k-tile's weights are zero and
    its ifmap is just the next 512 candidate columns (stride trick, no
    data duplication). For j==idx the 2nd k-tile carries the same-class
    mask: lhsT k1 = -448*onehot(class,row), rhs k1 = onehot(class,cand),
    so masking is completely free.
  - PSUM drain is the wall (only DVE and Act can read PSUM, 1 el/cycle,
    one PSUM operand per instruction; Pool/GPSIMD has no PSUM port and no
    max op at all; DMA cannot touch PSUM). Per (row-tile, j) pair of 4096
    candidates: Act drains 3584 els via Exp((v-c)/T) -> bf16 SBUF (the
    exp is free vs a copy and feeds the entropy directly: softmax over j
    of v/T == w / sum w for w = e^{(v-c)/T}); DVE drains the last 512 via
    a single segmented tensor_reduce(max, W=32) from PSUM.
  - Selection: group-max compression (G=32) -- DVE tensor_tensor max tree
    on the bf16 exp values at 2x (4 consumed els/cycle), 5 levels
    3584->112, plus the 16 reduced maxes (exp'd by Act) -> 128 leftover;
    then max8 -> match_replace -> max8 gives the sorted top-16 (monotone
    in v). Group-max loses a top-16 member only when two land in one
    group (~0.3% of slots, value shift ~1e-3): validated numerically at
    rel err ~3e-3 vs the fp32 reference (gate 2e-2).
  - Entropy: p=w/S; ent_k = A/S - lnS with A = sum_j w*ln(w). Pool does
    the j-sums (fp32 adds), Act does one batched Ln, DVE a short batched
    epilogue: (lnS - A/S)*decay summed over k -> partials [128, RT].
"""

import math
import sys

import numpy as np

for _p in ("/opt/trn_rl_repo",):
    if _p not in sys.path:
        sys.path.insert(0, _p)

N = 4096
D = 128
J = 4
NCORES = 8
NLOC = N // NCORES          # 512 rows per core
RT = NLOC // 128            # 4 row-tiles per core
K = 16
TEMP = 0.01
V = 0.95
MASK_NEG = -64.0            # exact in e4m3; dominates any cosine sim (+-448 = 0xFE decodes as NaN on the PE)
EXP_C = 0.35                # exp centering: w = exp((v - EXP_C)/TEMP)
NPAD = N + 512              # stride-trick tail pad for k-tile-1 reads

ASHARE = 3584               # candidates drained by Act per (t, j) pair
DSHARE = N - ASHARE         # candidates drained by DVE tensor_reduce
DW = 32                     # tensor_reduce window (G for the D share)
TREE_LVLS = (1792, 896, 448, 224, 112)   # A-share TT-max tree (G=32)
LO = TREE_LVLS[-1] + DSHARE // DW        # leftover per pair (128)

_BUILD_CACHE = {}
LAST_RESULT = None  # BassKernelResults of the most recent kernel() call


def _build(idx: int):
    if idx in _BUILD_CACHE:
        return _BUILD_CACHE[idx]

    import concourse.bacc as bacc
    import concourse.tile as tile
    import concourse.mybir as mybir

    f32 = mybir.dt.float32
    bf16 = mybir.dt.bfloat16
    fp8 = mybir.dt.float8e4
    AF = mybir.ActivationFunctionType
    OP = mybir.AluOpType
    DR = mybir.MatmulPerfMode.DoubleRow

    nc = bacc.Bacc(
        "TRN2",
        target_bir_lowering=False,
        debug=False,
        enable_asserts=False,
        num_devices=NCORES,
    )

    wPd = nc.dram_tensor("wP", [D, 2, NLOC], fp8, kind="ExternalInput").ap()
    wMd = nc.dram_tensor("wM", [D, 2, NLOC], fp8, kind="ExternalInput").ap()
    negsd = nc.dram_tensor("negsT", [J, D, NPAD], fp8,
                           kind="ExternalInput").ap()
    pkd = nc.dram_tensor("negsPK", [D, 2 * N], fp8, kind="ExternalInput").ap()
    decayd = nc.dram_tensor("decayW", [128, RT * K], f32,
                            kind="ExternalInput").ap()
    outd = nc.dram_tensor("out", [128, RT], f32, kind="ExternalOutput").ap()

    jorder = [j for j in range(J) if j != idx] + [idx]

    with tile.TileContext(nc) as tc:
        with (
            tc.tile_pool(name="consts", bufs=1) as cpool,
            tc.tile_pool(name="wb", bufs=2) as wpool,
            tc.tile_pool(name="tr1", bufs=2) as t1p,
            tc.tile_pool(name="tr2", bufs=2) as t2p,
            tc.tile_pool(name="tr3", bufs=2) as t3p,
            tc.tile_pool(name="tr4", bufs=2) as t4p,
            tc.tile_pool(name="lo", bufs=2) as lop,
            tc.tile_pool(name="dred", bufs=2) as drp,
            tc.tile_pool(name="rep", bufs=2) as repp,
            tc.tile_pool(name="ent", bufs=2) as epool,
            tc.tile_pool(name="psums", bufs=2, space="PSUM") as psp,
        ):
            # ---- loads ----
            negs_t = {}
            for j in range(J):
                if j != idx:
                    negs_t[j] = cpool.tile([128, NPAD], fp8, tag=f"negsT{j}",
                                           name=f"negsT{j}")
            pk_t = cpool.tile([128, 2 * N], fp8)
            wP_t = cpool.tile([128, 2, NLOC], fp8)
            wM_t = cpool.tile([128, 2, NLOC], fp8)
            decay_t = cpool.tile([128, RT * K], f32)

            j0 = jorder[0]
            # first unit's columns first so matmuls start early
            nc.sync.dma_start(negs_t[j0][:, 0:2560], negsd[j0, :, 0:2560])
            nc.scalar.dma_start(wP_t, wPd)
            nc.scalar.dma_start(wM_t, wMd)
            nc.scalar.dma_start(decay_t, decayd)
            nc.sync.dma_start(negs_t[j0][:, 2560:NPAD],
                              negsd[j0, :, 2560:NPAD])
            for j, eng in ((jorder[1], nc.gpsimd), (jorder[2], nc.sync)):
                if j == idx:
                    continue
                for h in range(2):
                    eng.dma_start(
                        negs_t[j][:, h * (NPAD // 2):(h + 1) * (NPAD // 2)],
                        negsd[j, :, h * (NPAD // 2):(h + 1) * (NPAD // 2)])
            for h, eng in ((0, nc.gpsimd), (1, nc.sync)):
                eng.dma_start(pk_t[:, h * N:(h + 1) * N],
                              pkd[:, h * N:(h + 1) * N])

            # warm the exp/ln activation table during the load phase
            bias_t = cpool.tile([128, 1], f32)
            nc.vector.memset(bias_t, -EXP_C / TEMP)
            warm = cpool.tile([128, 8], f32)
            nc.scalar.activation(out=warm, in_=decay_t[:, 0:8], func=AF.Exp)

            Vt_bf = cpool.tile([128, RT * J * K], bf16)   # sorted top-16 (w)
            Sall = cpool.tile([128, RT * K], f32)
            Aall = cpool.tile([128, RT * K], f32)

            pk_v = pk_t.rearrange("p (two n) -> p two n", two=2)

            for t in range(RT):
                Vt32 = epool.tile([128, J * K], f32, tag="Vt32",
                                  name=f"Vt32_{t}")
                for j in jorder:
                    lhsT = (wM_t if j == idx else wP_t)[
                        :, :, t * 128:(t + 1) * 128]
                    wbuf = wpool.tile([128, ASHARE], bf16, tag="wbuf")
                    dred = drp.tile([128, DSHARE // DW], f32, tag="dred")
                    lo = lop.tile([128, LO], bf16, tag="lo")

                    for h in range(2):
                        ps = psp.tile([128, 2048], f32, tag="sims")
                        for c in range(4):
                            c0 = h * 2048 + c * 512
                            if j == idx:
                                rhs = pk_v[:, :, c0:c0 + 512]
                            else:
                                rhs = negs_t[j][:, c0:c0 + 1024].rearrange(
                                    "p (two c) -> p two c", two=2)
                            nc.tensor.matmul(
                                ps[:, c * 512:(c + 1) * 512],
                                lhsT=lhsT, rhs=rhs,
                                start=True, stop=True, perf_mode=DR,
                            )
                        if h == 0:
                            nc.scalar.activation(
                                out=wbuf[:, 0:2048], in_=ps, func=AF.Exp,
                                scale=1.0 / TEMP, bias=bias_t)
                        else:
                            nc.scalar.activation(
                                out=wbuf[:, 2048:ASHARE], in_=ps[:, 0:1536],
                                func=AF.Exp,
                                scale=1.0 / TEMP, bias=bias_t)
                            nc.vector.tensor_reduce(
                                out=dred,
                                in_=ps[:, 1536:2048].rearrange(
                                    "p (g w) -> p g w", w=DW),
                                op=OP.max, axis=mybir.AxisListType.X)

                    # exp the DVE-reduced group maxes into the leftover buf
                    nc.scalar.activation(
                        out=lo[:, TREE_LVLS[-1]:LO], in_=dred, func=AF.Exp,
                        scale=1.0 / TEMP, bias=bias_t)

                    # DVE TT-max tree (bf16, 2x) 3584 -> 112 (G=32)
                    t1 = t1p.tile([128, 1792], bf16, tag="t1")
                    nc.vector.tensor_tensor(
                        t1, wbuf[:, 0:1792], wbuf[:, 1792:ASHARE], op=OP.max)
                    t2 = t2p.tile([128, 896], bf16, tag="t2")
                    nc.vector.tensor_tensor(
                        t2, t1[:, 0:896], t1[:, 896:1792], op=OP.max)
                    t3 = t3p.tile([128, 448], bf16, tag="t3")
                    nc.vector.tensor_tensor(
                        t3, t2[:, 0:448], t2[:, 448:896], op=OP.max)
                    t4 = t4p.tile([128, 224], bf16, tag="t4")
                    nc.vector.tensor_tensor(
                        t4, t3[:, 0:224], t3[:, 224:448], op=OP.max)
                    nc.vector.tensor_tensor(
                        lo[:, 0:112], t4[:, 0:112], t4[:, 112:224], op=OP.max)

                    # sorted top-16 of the 128 leftover group-maxes
                    vsl = Vt_bf[:, (t * J + j) * K:(t * J + j) * K + K]
                    nc.vector.max(out=vsl[:, 0:8], in_=lo)
                    rep = repp.tile([128, LO], bf16, tag="rep")
                    nc.vector.match_replace(
                        out=rep, in_to_replace=vsl[:, 0:8], in_values=lo,
                        imm_value=-1.0)
                    nc.vector.max(out=vsl[:, 8:16], in_=rep)

                # ---- per-row-tile entropy sums (fp32 on Pool) ----
                vt_src = Vt_bf[:, t * J * K:(t + 1) * J * K]
                nc.vector.tensor_copy(Vt32, vt_src)
                lnv = epool.tile([128, J * K], f32, tag="lnv",
                                 name=f"lnv_{t}")
                nc.scalar.activation(out=lnv, in_=vt_src, func=AF.Ln)
                s01 = epool.tile([128, K], f32, tag="s01", name=f"s01_{t}")
                s23 = epool.tile([128, K], f32, tag="s23", name=f"s23_{t}")
                nc.gpsimd.tensor_tensor(
                    s01, Vt32[:, 0:K], Vt32[:, K:2 * K], op=OP.add)
                nc.gpsimd.tensor_tensor(
                    s23, Vt32[:, 2 * K:3 * K], Vt32[:, 3 * K:4 * K],
                    op=OP.add)
                nc.gpsimd.tensor_tensor(
                    Sall[:, t * K:(t + 1) * K], s01, s23, op=OP.add)
                wl = epool.tile([128, J * K], f32, tag="wl", name=f"wl_{t}")
                nc.gpsimd.tensor_tensor(wl, Vt32, lnv, op=OP.mult)
                a01 = epool.tile([128, K], f32, tag="a01", name=f"a01_{t}")
                a23 = epool.tile([128, K], f32, tag="a23", name=f"a23_{t}")
                nc.gpsimd.tensor_tensor(
                    a01, wl[:, 0:K], wl[:, K:2 * K], op=OP.add)
                nc.gpsimd.tensor_tensor(
                    a23, wl[:, 2 * K:3 * K], wl[:, 3 * K:4 * K], op=OP.add)
                nc.gpsimd.tensor_tensor(
                    Aall[:, t * K:(t + 1) * K], a01, a23, op=OP.add)

            # ---- batched epilogue: negent = lnS - A/S; * decay; sum_k ----
            W = RT * K
            rS = cpool.tile([128, W], f32)
            nc.vector.reciprocal(rS, Sall)
            lnS = cpool.tile([128, W], f32)
            nc.scalar.activation(out=lnS, in_=Sall, func=AF.Ln)
            nc.vector.tensor_tensor(Aall, Aall, rS, op=OP.mult)   # A/S
            nc.vector.scalar_tensor_tensor(
                out=Aall, in0=lnS, scalar=1.0, in1=Aall,
                op0=OP.mult, op1=OP.subtract)                     # lnS - A/S
            nc.vector.tensor_tensor(Aall, Aall, decay_t, op=OP.mult)
            partials = cpool.tile([128, RT], f32)
            nc.vector.tensor_reduce(
                out=partials, in_=Aall.rearrange("p (t k) -> p t k", k=K),
                op=OP.add, axis=mybir.AxisListType.X)
            nc.sync.dma_start(outd, partials)

    nc.compile()
    _BUILD_CACHE[idx] = nc
    return nc


def kernel(feature, target, negative_features, idx):
    import ml_dtypes
    from concourse.bass_utils import run_bass_kernel_spmd

    npf8 = ml_dtypes.float8_e4m3fn

    feature = np.asarray(feature, dtype=np.float32)
    target = np.asarray(target).astype(np.int64)
    negs = np.asarray(negative_features, dtype=np.float32)
    idx_i = int(np.asarray(idx))

    # normalize + cast + transpose on host (layout/quantization prep)
    f = feature / np.maximum(
        np.linalg.norm(feature, axis=-1, keepdims=True), 1e-12)
    g = negs / np.maximum(
        np.linalg.norm(negs, axis=-1, keepdims=True), 1e-12)
    fT_all = np.ascontiguousarray(f.T.astype(npf8))                  # [D, N]
    negsT = g.transpose(0, 2, 1).astype(npf8)                        # [J,D,N]
    negsTp = np.zeros((J, D, NPAD), dtype=npf8)
    negsTp[:, :, 0:N] = negsT
    onehot = (target[None, :] == np.arange(J)[:, None])              # [J, N]
    pk = np.zeros((D, 2 * N), dtype=npf8)
    pk[:, 0:N] = negsT[idx_i]
    pk[0:J, N:2 * N] = onehot.astype(npf8)
    maskW = np.zeros((D, NLOC * NCORES), dtype=np.float32)
    for cls in range(J):
        maskW[cls, :] = MASK_NEG * (target == cls)
    decay = V ** np.arange(K, dtype=np.float64)
    decay = decay / decay.sum()
    decay_row = np.tile(decay.astype(np.float32), RT)                # [RT*K]
    decayW = np.broadcast_to(decay_row, (128, RT * K)).copy()

    nc = _build(idx_i)
    in_maps = []
    for c in range(NCORES):
        sl = slice(c * NLOC, (c + 1) * NLOC)
        wP = np.zeros((D, 2, NLOC), dtype=npf8)
        wP[:, 0, :] = fT_all[:, sl]
        wM = wP.copy()
        wM[:, 1, :] = maskW[:, sl].astype(npf8)
        in_maps.append({
            "wP": wP,
            "wM": wM,
            "negsT": negsTp,
            "negsPK": pk,
            "decayW": decayW,
        })

    res = run_bass_kernel_spmd(nc, in_maps, core_ids=list(range(NCORES)))
    global LAST_RESULT
    LAST_RESULT = res
    total = 0.0
    for c in range(NCORES):
        total += float(np.asarray(res.results[c]["out"],
                                  dtype=np.float64).sum())
    loss = -total / N + math.log(J)
    return np.float32(loss)


if __name__ == "__main__":
    rng = np.random.default_rng(0)
    f = rng.standard_normal((N, D)).astype(np.float32)
    ng = rng.standard_normal((J, N, D)).astype(np.float32)
    tg = rng.integers(0, J, size=N).astype(np.int64)
    print(kernel(f, tg, ng, 0))


# revision 8
# speedup vs baseline: 1.1660x; 1.0995x over previous
"""Trainium2 Bass kernel for nn_NegUniform (topk_masking) — v3.

Computes: L2-normalize feature & negative_features, sims = f_hat @ negs_hat^T
per negative set j (masked same-class for j==idx), top-16 per row, softmax
entropy over the J axis, decay-weighted mean + log(J).

Sharding: data-parallel over the n (row) dimension across 8 NeuronCores;
negatives/targets replicated. Each core returns per-row-tile partial sums
[128, RT]; the host reduces to the scalar.

Design (engine-measured rates drove every choice):
  - PE: fp8(e4m3) matmuls in DoubleRow perf mode (2 k-tiles summed per
    pass, 0.5 cyc/col). For j!=idx the 2nd k-tile's weights are zero and
    its ifmap is just the next 512 candidate columns (stride trick, no
    data duplication). For j==idx the 2nd k-tile carries the same-class
    mask: lhsT k1 = -448*onehot(class,row), rhs k1 = onehot(class,cand),
    so masking is completely free.
  - PSUM drain is the wall (only DVE and Act can read PSUM, 1 el/cycle,
    one PSUM operand per instruction; Pool/GPSIMD has no PSUM port and no
    max op at all; DMA cannot touch PSUM). Per (row-tile, j) pair of 4096
    candidates: Act drains 3584 els via Exp((v-c)/T) -> bf16 SBUF (the
    exp is free vs a copy and feeds the entropy directly: softmax over j
    of v/T == w / sum w for w = e^{(v-c)/T}); DVE drains the last 512 via
    a single segmented tensor_reduce(max, W=32) from PSUM.
  - Selection: group-max compression (G=32) -- DVE tensor_tensor max tree
    on the bf16 exp values at 2x (4 consumed els/cycle), 5 levels
    3584->112, plus the 16 reduced maxes (exp'd by Act) -> 128 leftover;
    then max8 -> match_replace -> max8 gives the sorted top-16 (monotone
    in v). Group-max loses a top-16 member only when two land in one
    group (~0.3% of slots, value shift ~1e-3): validated numerically at
    rel err ~3e-3 vs the fp32 reference (gate 2e-2).
  - Entropy: p=w/S; ent_k = A/S - lnS with A = sum_j w*ln(w). Pool does
    the j-sums (fp32 adds), Act does one batched Ln, DVE a short batched
    epilogue: (lnS - A/S)*decay summed over k -> partials [128, RT].
"""

import math
import sys

import numpy as np

for _p in ("/opt/trn_rl_repo",):
    if _p not in sys.path:
        sys.path.insert(0, _p)

N = 4096
D = 128
J = 4
NCORES = 8
NLOC = N // NCORES          # 512 rows per core
RT = NLOC // 128            # 4 row-tiles per core
K = 16
TEMP = 0.01
V = 0.95
MASK_NEG = -64.0            # exact in e4m3; dominates any cosine sim (+-448 = 0xFE decodes as NaN on the PE)
EXP_C = 0.35                # exp centering: w = exp((v - EXP_C)/TEMP)
NPAD = N + 512              # stride-trick tail pad for k-tile-1 reads

DSHARE = 512                # candidates drained by DVE tensor_reduce
ASHARE = N - DSHARE         # candidates drained by Act per (t, j) pair
DW = 32                     # tensor_reduce window (G for the D share)
NLVL = 5                    # A-share TT-max tree levels (G=32)
TREE_LVLS = tuple(ASHARE >> (i + 1) for i in range(NLVL))
LO = TREE_LVLS[-1] + DSHARE // DW        # leftover per pair

_BUILD_CACHE = {}
LAST_RESULT = None  # BassKernelResults of the most recent kernel() call


def _build(idx: int):
    if idx in _BUILD_CACHE:
        return _BUILD_CACHE[idx]

    import concourse.bacc as bacc
    import concourse.tile as tile
    import concourse.mybir as mybir

    f32 = mybir.dt.float32
    bf16 = mybir.dt.bfloat16
    fp8 = mybir.dt.float8e4
    AF = mybir.ActivationFunctionType
    OP = mybir.AluOpType
    DR = mybir.MatmulPerfMode.DoubleRow

    nc = bacc.Bacc(
        "TRN2",
        target_bir_lowering=False,
        debug=False,
        enable_asserts=False,
        num_devices=NCORES,
    )

    wPd = nc.dram_tensor("wP", [D, 2, NLOC], fp8, kind="ExternalInput").ap()
    wMd = nc.dram_tensor("wM", [D, 2, NLOC], fp8, kind="ExternalInput").ap()
    negsd = nc.dram_tensor("negsT", [J, D, NPAD], fp8,
                           kind="ExternalInput").ap()
    pkd = nc.dram_tensor("negsPK", [D, 2 * N], fp8, kind="ExternalInput").ap()
    decayd = nc.dram_tensor("decayW", [128, RT * K], f32,
                            kind="ExternalInput").ap()
    outd = nc.dram_tensor("out", [128, RT], f32, kind="ExternalOutput").ap()

    jorder = [j for j in range(J) if j != idx] + [idx]

    with tile.TileContext(nc) as tc:
        with (
            tc.tile_pool(name="consts", bufs=1) as cpool,
            tc.tile_pool(name="wb", bufs=3) as wpool,
            tc.tile_pool(name="tr1", bufs=2) as t1p,
            tc.tile_pool(name="tr2", bufs=2) as t2p,
            tc.tile_pool(name="tr3", bufs=2) as t3p,
            tc.tile_pool(name="tr4", bufs=2) as t4p,
            tc.tile_pool(name="lo", bufs=3) as lop,
            tc.tile_pool(name="dred", bufs=2) as drp,
            tc.tile_pool(name="rep", bufs=2) as repp,
            tc.tile_pool(name="ent", bufs=2) as epool,
            tc.tile_pool(name="psums", bufs=2, space="PSUM") as psp,
        ):
            # ---- loads ----
            negs_t = {}
            for j in range(J):
                if j != idx:
                    negs_t[j] = cpool.tile([128, NPAD], fp8, tag=f"negsT{j}",
                                           name=f"negsT{j}")
            pk_t = cpool.tile([128, 2 * N], fp8)
            wP_t = cpool.tile([128, 2, NLOC], fp8)
            wM_t = cpool.tile([128, 2, NLOC], fp8)
            decay_t = cpool.tile([128, RT * K], f32)

            j0 = jorder[0]
            # first unit's columns first so matmuls start early
            nc.sync.dma_start(negs_t[j0][:, 0:2560], negsd[j0, :, 0:2560])
            nc.scalar.dma_start(wP_t, wPd)
            nc.scalar.dma_start(wM_t, wMd)
            nc.scalar.dma_start(decay_t, decayd)
            nc.sync.dma_start(negs_t[j0][:, 2560:NPAD],
                              negsd[j0, :, 2560:NPAD])
            for j, eng in ((jorder[1], nc.gpsimd), (jorder[2], nc.sync)):
                if j == idx:
                    continue
                for h in range(2):
                    eng.dma_start(
                        negs_t[j][:, h * (NPAD // 2):(h + 1) * (NPAD // 2)],
                        negsd[j, :, h * (NPAD // 2):(h + 1) * (NPAD // 2)])
            for h, eng in ((0, nc.gpsimd), (1, nc.sync)):
                eng.dma_start(pk_t[:, h * N:(h + 1) * N],
                              pkd[:, h * N:(h + 1) * N])

            # warm the exp/ln activation table during the load phase
            bias_t = cpool.tile([128, 1], f32)
            nc.vector.memset(bias_t, -EXP_C / TEMP)
            warm = cpool.tile([128, 8], f32)
            nc.scalar.activation(out=warm, in_=decay_t[:, 0:8], func=AF.Exp)

            Vt_bf = cpool.tile([128, RT * J * K], bf16)   # sorted top-16 (w)
            Sall = cpool.tile([128, RT * K], f32)
            Aall = cpool.tile([128, RT * K], f32)
            Vt32_all = {}

            pk_v = pk_t.rearrange("p (two n) -> p two n", two=2)

            for t in range(RT):
                Vt32 = cpool.tile([128, J * K], f32, tag=f"Vt32_{t}",
                                  name=f"Vt32_{t}")
                for j in jorder:
                    lhsT = (wM_t if j == idx else wP_t)[
                        :, :, t * 128:(t + 1) * 128]
                    wbuf = wpool.tile([128, ASHARE], bf16, tag="wbuf")
                    dred = drp.tile([128, DSHARE // DW], f32, tag="dred")
                    lo = lop.tile([128, LO], bf16, tag="lo")

                    for h in range(2):
                        ps = psp.tile([128, 2048], f32, tag="sims")
                        for c in range(4):
                            c0 = h * 2048 + c * 512
                            if j == idx:
                                rhs = pk_v[:, :, c0:c0 + 512]
                            else:
                                rhs = negs_t[j][:, c0:c0 + 1024].rearrange(
                                    "p (two c) -> p two c", two=2)
                            nc.tensor.matmul(
                                ps[:, c * 512:(c + 1) * 512],
                                lhsT=lhsT, rhs=rhs,
                                start=True, stop=True, perf_mode=DR,
                            )
                        if h == 0:
                            nc.scalar.activation(
                                out=wbuf[:, 0:2048], in_=ps, func=AF.Exp,
                                scale=1.0 / TEMP, bias=bias_t)
                        else:
                            nc.scalar.activation(
                                out=wbuf[:, 2048:ASHARE], in_=ps[:, 0:1536],
                                func=AF.Exp,
                                scale=1.0 / TEMP, bias=bias_t)
                            nc.vector.tensor_reduce(
                                out=dred,
                                in_=ps[:, 1536:2048].rearrange(
                                    "p (g w) -> p g w", w=DW),
                                op=OP.max, axis=mybir.AxisListType.X)

                    # exp the DVE-reduced group maxes into the leftover buf
                    nc.scalar.activation(
                        out=lo[:, TREE_LVLS[-1]:LO], in_=dred, func=AF.Exp,
                        scale=1.0 / TEMP, bias=bias_t)

                    # DVE TT-max tree (bf16, 2x) 3584 -> 112 (G=32)
                    t1 = t1p.tile([128, 1792], bf16, tag="t1")
                    nc.vector.tensor_tensor(
                        t1, wbuf[:, 0:1792], wbuf[:, 1792:ASHARE], op=OP.max)
                    t2 = t2p.tile([128, 896], bf16, tag="t2")
                    nc.vector.tensor_tensor(
                        t2, t1[:, 0:896], t1[:, 896:1792], op=OP.max)
                    t3 = t3p.tile([128, 448], bf16, tag="t3")
                    nc.vector.tensor_tensor(
                        t3, t2[:, 0:448], t2[:, 448:896], op=OP.max)
                    t4 = t4p.tile([128, 224], bf16, tag="t4")
                    nc.vector.tensor_tensor(
                        t4, t3[:, 0:224], t3[:, 224:448], op=OP.max)
                    nc.vector.tensor_tensor(
                        lo[:, 0:112], t4[:, 0:112], t4[:, 112:224], op=OP.max)

                    # sorted top-16 of the 128 leftover group-maxes
                    vsl = Vt_bf[:, (t * J + j) * K:(t * J + j) * K + K]
                    nc.vector.max(out=vsl[:, 0:8], in_=lo)
                    rep = repp.tile([128, LO], bf16, tag="rep")
                    nc.vector.match_replace(
                        out=rep, in_to_replace=vsl[:, 0:8], in_values=lo,
                        imm_value=-1.0)
                    nc.vector.max(out=vsl[:, 8:16], in_=rep)

                # ---- per-row-tile S sums (fp32 on Pool; no Ln needed) ----
                vt_src = Vt_bf[:, t * J * K:(t + 1) * J * K]
                nc.vector.tensor_copy(Vt32, vt_src)
                s01 = epool.tile([128, K], f32, tag="s01", name=f"s01_{t}")
                s23 = epool.tile([128, K], f32, tag="s23", name=f"s23_{t}")
                nc.gpsimd.tensor_tensor(
                    s01, Vt32[:, 0:K], Vt32[:, K:2 * K], op=OP.add)
                nc.gpsimd.tensor_tensor(
                    s23, Vt32[:, 2 * K:3 * K], Vt32[:, 3 * K:4 * K],
                    op=OP.add)
                nc.gpsimd.tensor_tensor(
                    Sall[:, t * K:(t + 1) * K], s01, s23, op=OP.add)
                Vt32_all[t] = Vt32

            # ---- deferred Ln phase (single activation-table switch) ----
            W = RT * J * K
            lnv_all = cpool.tile([128, W], f32)
            nc.scalar.activation(out=lnv_all, in_=Vt_bf, func=AF.Ln)
            for t in range(RT):
                wl = epool.tile([128, J * K], f32, tag="wl", name=f"wl_{t}")
                nc.vector.tensor_tensor(
                    wl, Vt32_all[t],
                    lnv_all[:, t * J * K:(t + 1) * J * K], op=OP.mult)
                a01 = epool.tile([128, K], f32, tag="a01", name=f"a01_{t}")
                a23 = epool.tile([128, K], f32, tag="a23", name=f"a23_{t}")
                nc.gpsimd.tensor_tensor(
                    a01, wl[:, 0:K], wl[:, K:2 * K], op=OP.add)
                nc.gpsimd.tensor_tensor(
                    a23, wl[:, 2 * K:3 * K], wl[:, 3 * K:4 * K], op=OP.add)
                nc.gpsimd.tensor_tensor(
                    Aall[:, t * K:(t + 1) * K], a01, a23, op=OP.add)

            # ---- batched epilogue: negent = lnS - A/S; * decay; sum_k ----
            W = RT * K
            rS = cpool.tile([128, W], f32)
            nc.vector.reciprocal(rS, Sall)
            lnS = cpool.tile([128, W], f32)
            nc.scalar.activation(out=lnS, in_=Sall, func=AF.Ln)
            nc.vector.tensor_tensor(Aall, Aall, rS, op=OP.mult)   # A/S
            nc.vector.scalar_tensor_tensor(
                out=Aall, in0=lnS, scalar=1.0, in1=Aall,
                op0=OP.mult, op1=OP.subtract)                     # lnS - A/S
            nc.vector.tensor_tensor(Aall, Aall, decay_t, op=OP.mult)
            partials = cpool.tile([128, RT], f32)
            nc.vector.tensor_reduce(
                out=partials, in_=Aall.rearrange("p (t k) -> p t k", k=K),
                op=OP.add, axis=mybir.AxisListType.X)
            nc.sync.dma_start(outd, partials)

    nc.compile()
    _BUILD_CACHE[idx] = nc
    return nc


def kernel(feature, target, negative_features, idx):
    import ml_dtypes
    from concourse.bass_utils import run_bass_kernel_spmd

    npf8 = ml_dtypes.float8_e4m3fn

    feature = np.asarray(feature, dtype=np.float32)
    target = np.asarray(target).astype(np.int64)
    negs = np.asarray(negative_features, dtype=np.float32)
    idx_i = int(np.asarray(idx))

    # normalize + cast + transpose on host (layout/quantization prep)
    f = feature / np.maximum(
        np.linalg.norm(feature, axis=-1, keepdims=True), 1e-12)
    g = negs / np.maximum(
        np.linalg.norm(negs, axis=-1, keepdims=True), 1e-12)
    fT_all = np.ascontiguousarray(f.T.astype(npf8))                  # [D, N]
    negsT = g.transpose(0, 2, 1).astype(npf8)                        # [J,D,N]
    negsTp = np.zeros((J, D, NPAD), dtype=npf8)
    negsTp[:, :, 0:N] = negsT
    onehot = (target[None, :] == np.arange(J)[:, None])              # [J, N]
    pk = np.zeros((D, 2 * N), dtype=npf8)
    pk[:, 0:N] = negsT[idx_i]
    pk[0:J, N:2 * N] = onehot.astype(npf8)
    maskW = np.zeros((D, NLOC * NCORES), dtype=np.float32)
    for cls in range(J):
        maskW[cls, :] = MASK_NEG * (target == cls)
    decay = V ** np.arange(K, dtype=np.float64)
    decay = decay / decay.sum()
    decay_row = np.tile(decay.astype(np.float32), RT)                # [RT*K]
    decayW = np.broadcast_to(decay_row, (128, RT * K)).copy()

    nc = _build(idx_i)
    in_maps = []
    for c in range(NCORES):
        sl = slice(c * NLOC, (c + 1) * NLOC)
        wP = np.zeros((D, 2, NLOC), dtype=npf8)
        wP[:, 0, :] = fT_all[:, sl]
        wM = wP.copy()
        wM[:, 1, :] = maskW[:, sl].astype(npf8)
        in_maps.append({
            "wP": wP,
            "wM": wM,
            "negsT": negsTp,
            "negsPK": pk,
            "decayW": decayW,
        })

    res = run_bass_kernel_spmd(nc, in_maps, core_ids=list(range(NCORES)))
    global LAST_RESULT
    LAST_RESULT = res
    total = 0.0
    for c in range(NCORES):
        total += float(np.asarray(res.results[c]["out"],
                                  dtype=np.float64).sum())
    loss = -total / N + math.log(J)
    return np.float32(loss)


if __name__ == "__main__":
    rng = np.random.default_rng(0)
    f = rng.standard_normal((N, D)).astype(np.float32)
    ng = rng.standard_normal((J, N, D)).astype(np.float32)
    tg = rng.integers(0, J, size=N).astype(np.int64)
    print(kernel(f, tg, ng, 0))
